# revision 2
# baseline (speedup 1.0000x reference)
"""Trainium2 Bass kernel for nn_BasicTransformerBlock (cross-attention block).

Reference computation (per batch b of 16):
  q = x[b] @ Wq                        [4096, 512]
  k/v   = ctx_txt[b] @ Wk/Wv           [77, 512]
  k/v_ip= ctx_img[b] @ Wk_ip/Wv_ip     [16, 512]
  per head h (8 heads, d=64):
    sim  = q_h @ k_h.T * 0.125         softmax over keys (txt and img separately)
    out  = text_scale * softmax(sim_txt) @ v_txt + img_scale * softmax(sim_img) @ v_img
  out = merge_heads(out) @ Wo + bo     [4096, 320]

Sharding: data-parallel over batch, 2 batches per core on 8 cores.

Layout notes (per core):
  - Everything transposed-for-PE via DMA transpose (bf16, XBAR 16x128 tiles).
  - Keys padded: txt keys at partitions/cols 0:77, img keys at 96:112
    (PE partition accesses must start at 0/32/64; DMA handles offset 96).
  - All matmul operands bf16, f32 PSUM accumulation.
  - Softmax skips max-subtraction (|sim*0.125| is O(1) for this problem);
    normalization + text/img scales folded into one scalar_tensor_tensor
    on the probabilities in token-partition layout.
"""
import sys

if "/opt/trn_rl_repo" not in sys.path:
    sys.path.insert(0, "/opt/trn_rl_repo")

import numpy as np

import concourse.bacc as bacc
import concourse.mybir as mybir
import concourse.tile as tile
from concourse.bass_utils import run_bass_kernel_spmd

F32 = mybir.dt.float32
BF16 = mybir.dt.bfloat16
AF = mybir.ActivationFunctionType
ALU = mybir.AluOpType

N_CORES = 8
B = 16
BPC = B // N_CORES          # batches per core
N = 4096                    # tokens
QD = 320                    # query dim
CD = 1024                   # context dim
H = 8                       # heads
D = 64                      # head dim
ID = H * D                  # 512
TXT = 77                    # text keys
IMG = 16                    # image keys
IMG0 = 96                   # partition/col offset of img keys (32-aligned)
KSPAN = IMG0 + IMG          # 112: used key span
NCH = N // 128              # 32 token chunks
SCALE = 0.125               # 1/sqrt(64)

_NC_CACHE = None


def _build_nc():
    nc = bacc.Bacc("TRN2", target_bir_lowering=False, debug=False)

    x = nc.dram_tensor("x", [BPC, N, QD], F32, kind="ExternalInput").ap()
    ctx = nc.dram_tensor("context", [BPC, 93, CD], F32, kind="ExternalInput").ap()
    Wq = nc.dram_tensor("Wq", [QD, ID], F32, kind="ExternalInput").ap()
    Wk = nc.dram_tensor("Wk", [CD, ID], F32, kind="ExternalInput").ap()
    Wv = nc.dram_tensor("Wv", [CD, ID], F32, kind="ExternalInput").ap()
    Wk_ip = nc.dram_tensor("Wk_ip", [CD, ID], F32, kind="ExternalInput").ap()
    Wv_ip = nc.dram_tensor("Wv_ip", [CD, ID], F32, kind="ExternalInput").ap()
    Wo = nc.dram_tensor("Wo", [ID, QD], F32, kind="ExternalInput").ap()
    bo = nc.dram_tensor("bo", [QD], F32, kind="ExternalInput").ap()
    tscale = nc.dram_tensor("text_scale", [1], F32, kind="ExternalInput").ap()
    iscale = nc.dram_tensor("img_scale", [1], F32, kind="ExternalInput").ap()
    out = nc.dram_tensor("out", [BPC, N, QD], F32, kind="ExternalOutput").ap()

    with tile.TileContext(nc) as tc:
        with tc.tile_pool(name="wpool", bufs=1) as wpool, \
             tc.tile_pool(name="stage", bufs=2) as stage, \
             tc.tile_pool(name="xpool", bufs=3) as xpool, \
             tc.tile_pool(name="kvpool", bufs=2) as kvpool, \
             tc.tile_pool(name="bigpool", bufs=1) as bigpool, \
             tc.tile_pool(name="appool", bufs=3) as appool, \
             tc.tile_pool(name="opool", bufs=3) as opool, \
             tc.tile_pool(name="pp", bufs=2, space="PSUM") as pp:

            # ---------------- weight prep (bf16 casts) ----------------
            def load_cast(dram_ap, kt_count, mdim, name):
                st = stage.tile([128, kt_count, mdim], F32, tag="wstage",
                                name=f"st_{name}")
                nc.sync.dma_start(
                    out=st[:],
                    in_=dram_ap.rearrange("(k p) m -> p k m", p=128))
                wbf = wpool.tile([128, kt_count, mdim], BF16, name=f"w_{name}")
                nc.vector.tensor_copy(wbf[:], st[:])
                return wbf

            wk = load_cast(Wk, 8, ID, "wk")
            wv = load_cast(Wv, 8, ID, "wv")
            wkip = load_cast(Wk_ip, 8, ID, "wkip")
            wvip = load_cast(Wv_ip, 8, ID, "wvip")
            wo = load_cast(Wo, 4, QD, "wo")

            # Wq [320, 512]: k-tiles 128/128/64
            st_wq = stage.tile([128, 3, ID], F32, tag="wstage")
            nc.sync.dma_start(
                out=st_wq[:, 0:2, :],
                in_=Wq[0:256, :].rearrange("(k p) m -> p k m", p=128))
            nc.sync.dma_start(out=st_wq[0:64, 2, :], in_=Wq[256:320, :])
            wq = wpool.tile([128, 3, ID], BF16)
            nc.vector.tensor_copy(wq[:, 0:2, :], st_wq[:, 0:2, :])
            nc.vector.tensor_copy(wq[0:64, 2, :], st_wq[0:64, 2, :])

            bo_f32 = wpool.tile([1, QD], F32)
            nc.sync.dma_start(out=bo_f32[:], in_=bo[None, :])
            bo_bf = wpool.tile([1, QD], BF16)
            nc.vector.tensor_copy(bo_bf[:], bo_f32[:])
            ones1 = wpool.tile([1, 128], BF16)
            nc.gpsimd.memset(ones1[:], 1.0)

            ts_sb = wpool.tile([1, 1], F32)
            nc.sync.dma_start(out=ts_sb[:], in_=tscale[:, None])
            is_sb = wpool.tile([1, 1], F32)
            nc.sync.dma_start(out=is_sb[:], in_=iscale[:, None])
            ts_col = wpool.tile([128, 1], F32)
            nc.gpsimd.partition_broadcast(ts_col[:], ts_sb[:])
            is_col = wpool.tile([128, 1], F32)
            nc.gpsimd.partition_broadcast(is_col[:], is_sb[:])

            for b in range(BPC):
                # ---------------- context -> K^T, V ----------------
                ctx_f32 = kvpool.tile([128, CD], F32)
                nc.gpsimd.memset(ctx_f32[:], 0.0)
                nc.sync.dma_start(out=ctx_f32[0:TXT, :], in_=ctx[b, 0:TXT, :])
                nc.sync.dma_start(out=ctx_f32[IMG0:KSPAN, :], in_=ctx[b, TXT:93, :])
                ctx_bf = kvpool.tile([128, CD], BF16)
                nc.vector.tensor_copy(ctx_bf[:], ctx_f32[:])
                ctxt = kvpool.tile([128, 8, 128], BF16)
                nc.sync.dma_start(out=ctxt[:], in_=ctx_bf[:], transpose=True)

                # K^T  [512 (4 m-tiles), keys]
                psum_kt = pp.tile([128, 4, 128], F32, tag="proj", bufs=1)
                for m in range(4):
                    for k in range(8):
                        nc.tensor.matmul(
                            psum_kt[:, m, 0:TXT],
                            wk[:, k, 128 * m:128 * (m + 1)],
                            ctxt[:, k, 0:TXT],
                            start=(k == 0), stop=(k == 7))
                for m in range(4):
                    for k in range(8):
                        nc.tensor.matmul(
                            psum_kt[:, m, IMG0:KSPAN],
                            wkip[:, k, 128 * m:128 * (m + 1)],
                            ctxt[:, k, IMG0:KSPAN],
                            start=(k == 0), stop=(k == 7))
                kt = kvpool.tile([128, 4, 128], BF16)
                nc.gpsimd.memset(kt[:], 0.0)
                nc.vector.tensor_copy(kt[:, :, 0:TXT], psum_kt[:, :, 0:TXT])
                nc.vector.tensor_copy(kt[:, :, IMG0:KSPAN],
                                      psum_kt[:, :, IMG0:KSPAN])

                # V [keys, 512]
                psum_vt = pp.tile([128, ID], F32, tag="proj", bufs=1)
                for k in range(8):
                    nc.tensor.matmul(
                        psum_vt[0:TXT, :], ctxt[:, k, 0:TXT], wv[:, k, :],
                        start=(k == 0), stop=(k == 7))
                psum_vi = pp.tile([16, ID], F32, tag="proj_small", bufs=1)
                for k in range(8):
                    nc.tensor.matmul(
                        psum_vi[:, :], ctxt[:, k, IMG0:KSPAN], wvip[:, k, :],
                        start=(k == 0), stop=(k == 7))
                v_sb = kvpool.tile([128, ID], BF16)
                nc.gpsimd.memset(v_sb[:], 0.0)
                nc.vector.tensor_copy(v_sb[0:TXT, :], psum_vt[0:TXT, :])
                v_img = kvpool.tile([16, ID], BF16)
                nc.vector.tensor_copy(v_img[:], psum_vi[:])
                nc.sync.dma_start(out=v_sb[IMG0:KSPAN, :], in_=v_img[:])

                # ---------------- x -> x^T (bf16) ----------------
                xt = bigpool.tile([128, 3, N], BF16, tag="xt")
                for c in range(NCH):
                    x_f32 = xpool.tile([128, QD], F32)
                    nc.sync.dma_start(
                        out=x_f32[:], in_=x[b, 128 * c:128 * (c + 1), :])
                    x_bf = xpool.tile([128, 384], BF16)
                    nc.vector.tensor_copy(x_bf[:, 0:QD], x_f32[:])
                    nc.sync.dma_start(
                        out=xt[:, :, 128 * c:128 * (c + 1)],
                        in_=x_bf[:], transpose=True)

                # ---------------- Q^T = Wq.T @ x^T ----------------
                qt = bigpool.tile([128, 4, N], BF16, tag="qt")
                for m in range(4):
                    for n in range(8):
                        psum_q = pp.tile([128, 512], F32, tag="qproj", bufs=2)
                        for ki, kp in enumerate((128, 128, 64)):
                            nc.tensor.matmul(
                                psum_q[:],
                                wq[0:kp, ki, 128 * m:128 * (m + 1)],
                                xt[0:kp, ki, 512 * n:512 * (n + 1)],
                                start=(ki == 0), stop=(ki == 2))
                        nc.scalar.activation(
                            qt[:, m, 512 * n:512 * (n + 1)], psum_q[:], AF.Copy)

                # ---------------- attention ----------------
                comb = bigpool.tile([128, 4, N], BF16, tag="comb")
                for hp in range(4):          # head pairs
                    dsum = appool.tile([128, 2, 2, NCH], F32, tag="dsum")
                    rsum = appool.tile([128, 2, 2, NCH], F32, tag="rsum")
                    for g in range(8):       # groups of 4 token chunks
                        probs = appool.tile([128, 2, 4, 128], BF16, tag="probs")
                        for hh in range(2):
                            h = 2 * hp + hh
                            psum_s = pp.tile([128, 4, 128], F32, tag="sim", bufs=2)
                            for c4 in range(4):
                                tok = 128 * (4 * g + c4)
                                nc.tensor.matmul(
                                    psum_s[:, c4, 0:KSPAN],
                                    qt[64 * hh:64 * (hh + 1), hp,
                                       tok:tok + 128],
                                    kt[64 * hh:64 * (hh + 1), hp, 0:KSPAN],
                                    start=True, stop=True)
                            nc.scalar.activation(
                                probs[:, hh, :, 0:KSPAN],
                                psum_s[:, :, 0:KSPAN], AF.Exp, scale=SCALE)
                            nc.vector.reduce_sum(
                                out=dsum[:, hh, 0, 4 * g:4 * (g + 1)],
                                in_=probs[:, hh, :, 0:TXT],
                                axis=mybir.AxisListType.X)
                            nc.vector.reduce_sum(
                                out=dsum[:, hh, 1, 4 * g:4 * (g + 1)],
                                in_=probs[:, hh, :, IMG0:KSPAN],
                                axis=mybir.AxisListType.X)
                            nc.vector.reciprocal(
                                rsum[:, hh, :, 4 * g:4 * (g + 1)],
                                dsum[:, hh, :, 4 * g:4 * (g + 1)])
                            nc.vector.scalar_tensor_tensor(
                                out=probs[:, hh, :, 0:TXT],
                                in0=probs[:, hh, :, 0:TXT],
                                scalar=ts_col[:, 0:1],
                                in1=rsum[:, hh, 0, 4 * g:4 * (g + 1)][:, :, None]
                                    .broadcast_to([128, 4, TXT]),
                                op0=ALU.mult, op1=ALU.mult)
                            nc.vector.scalar_tensor_tensor(
                                out=probs[:, hh, :, IMG0:KSPAN],
                                in0=probs[:, hh, :, IMG0:KSPAN],
                                scalar=is_col[:, 0:1],
                                in1=rsum[:, hh, 1, 4 * g:4 * (g + 1)][:, :, None]
                                    .broadcast_to([128, 4, IMG]),
                                op0=ALU.mult, op1=ALU.mult)
                        probsT = appool.tile([128, 8, 128], BF16, tag="probsT")
                        nc.sync.dma_start(
                            out=probsT[:],
                            in_=probs.rearrange("p a b k -> p (a b k)"),
                            transpose=True)
                        psum_pv = pp.tile([128, 512], F32, tag="pv", bufs=1)
                        for hh in range(2):
                            h = 2 * hp + hh
                            nc.tensor.matmul(
                                psum_pv[64 * hh:64 * (hh + 1), :],
                                v_sb[0:KSPAN, 64 * h:64 * (h + 1)],
                                probsT[0:KSPAN, 4 * hh:4 * (hh + 1), :],
                                start=True, stop=True)
                        nc.vector.tensor_copy(
                            comb[:, hp, 512 * g:512 * (g + 1)], psum_pv[:])

                # ---------------- out projection ----------------
                for c in range(NCH):
                    psum_o = pp.tile([128, QD], F32, tag="oproj", bufs=1)
                    for k in range(4):
                        nc.tensor.matmul(
                            psum_o[:],
                            comb[:, k, 128 * c:128 * (c + 1)],
                            wo[:, k, :],
                            start=(k == 0), stop=False)
                    nc.tensor.matmul(
                        psum_o[:], ones1[:, :], bo_bf[:, :],
                        start=False, stop=True)
                    out_sb = opool.tile([128, QD], F32)
                    nc.scalar.activation(out_sb[:], psum_o[:], AF.Copy)
                    nc.sync.dma_start(
                        out=out[b, 128 * c:128 * (c + 1), :], in_=out_sb[:])

    nc.compile()
    return nc


def _get_nc():
    global _NC_CACHE
    if _NC_CACHE is None:
        _NC_CACHE = _build_nc()
    return _NC_CACHE


def kernel(x, context, Wq, Wk, Wv, Wk_ip, Wv_ip, Wo, bo, text_scale, img_scale):
    x = np.ascontiguousarray(np.asarray(x, dtype=np.float32))
    context = np.ascontiguousarray(np.asarray(context, dtype=np.float32))
    shared = {
        "Wq": np.asarray(Wq, np.float32), "Wk": np.asarray(Wk, np.float32),
        "Wv": np.asarray(Wv, np.float32), "Wk_ip": np.asarray(Wk_ip, np.float32),
        "Wv_ip": np.asarray(Wv_ip, np.float32), "Wo": np.asarray(Wo, np.float32),
        "bo": np.asarray(bo, np.float32),
        "text_scale": np.asarray(text_scale, np.float32),
        "img_scale": np.asarray(img_scale, np.float32),
    }
    nc = _get_nc()
    in_maps = []
    for c in range(N_CORES):
        m = dict(shared)
        m["x"] = x[BPC * c:BPC * (c + 1)]
        m["context"] = context[BPC * c:BPC * (c + 1)]
        in_maps.append(m)
    res = run_bass_kernel_spmd(nc, in_maps, core_ids=list(range(N_CORES)))
    return np.concatenate([res.results[c]["out"] for c in range(N_CORES)], axis=0)


# revision 10
# speedup vs baseline: 1.4487x; 1.4487x over previous
"""Trainium2 Bass kernel for nn_BasicTransformerBlock (cross-attention block).

Reference computation (per batch b of 16):
  q = x[b] @ Wq                        [4096, 512]
  k/v    = ctx_txt[b] @ Wk/Wv          [77, 512]
  k/v_ip = ctx_img[b] @ Wk_ip/Wv_ip    [16, 512]
  per head h (8 heads, d=64):
    sim = q_h @ k_h.T * 0.125, softmax over keys (txt / img separately)
    out_h = ts * softmax(sim_txt) @ v_txt + is * softmax(sim_img) @ v_img
  out = merge_heads(out) @ Wo + bo     [4096, 320]

Sharding: data-parallel over batch, 2 batches per core on 8 cores.

Layout (per core):
  - All PE operands bf16 (f32 PSUM accumulation); transposes via DMA xbar.
  - Keys padded: txt keys at partitions/cols 0:77, img keys at 96:112
    (PE partition bases must be 0/32/64; DMA places the img segment).
  - Softmax skips max-subtraction (|sim|*0.125 is O(1) here); normalization
    and the text/img output scales fold into one scalar_tensor_tensor in
    token-partition layout.
  - DMA ring split: SP ring = xbar transposes only; ACT ring = HBM loads and
    stores; SWDGE (gpsimd) = one-time weight loads.
"""
import sys

if "/opt/trn_rl_repo" not in sys.path:
    sys.path.insert(0, "/opt/trn_rl_repo")

import numpy as np

import concourse.bacc as bacc
import concourse.mybir as mybir
import concourse.tile as tile
from concourse.bass_utils import run_bass_kernel_spmd

F32 = mybir.dt.float32
BF16 = mybir.dt.bfloat16
AF = mybir.ActivationFunctionType
ALU = mybir.AluOpType
X_AX = mybir.AxisListType.X

N_CORES = 8
B = 16
BPC = B // N_CORES          # batches per core
N = 4096                    # tokens
QD = 320                    # query dim
CD = 1024                   # context dim
H = 8                       # heads
D = 64                      # head dim
ID = H * D                  # 512
TXT = 77                    # text keys
IMG = 16                    # image keys
IMG0 = 96                   # partition/col offset of img keys (32-aligned)
KSPAN = IMG0 + IMG          # 112
NCH = N // 128              # 32 token chunks
NG = NCH // 4               # 8 groups of 4 chunks
SCALE = 0.125               # 1/sqrt(64)

_NC_CACHE = None


def _build_nc():
    nc = bacc.Bacc("TRN2", target_bir_lowering=False, debug=False)

    x = nc.dram_tensor("x", [BPC, N, QD], F32, kind="ExternalInput").ap()
    ctx = nc.dram_tensor("context", [BPC, 93, CD], F32, kind="ExternalInput").ap()
    Wq = nc.dram_tensor("Wq", [QD, ID], F32, kind="ExternalInput").ap()
    Wk = nc.dram_tensor("Wk", [CD, ID], F32, kind="ExternalInput").ap()
    Wv = nc.dram_tensor("Wv", [CD, ID], F32, kind="ExternalInput").ap()
    Wk_ip = nc.dram_tensor("Wk_ip", [CD, ID], F32, kind="ExternalInput").ap()
    Wv_ip = nc.dram_tensor("Wv_ip", [CD, ID], F32, kind="ExternalInput").ap()
    Wo = nc.dram_tensor("Wo", [ID, QD], F32, kind="ExternalInput").ap()
    bo = nc.dram_tensor("bo", [QD], F32, kind="ExternalInput").ap()
    tscale = nc.dram_tensor("text_scale", [1], F32, kind="ExternalInput").ap()
    iscale = nc.dram_tensor("img_scale", [1], F32, kind="ExternalInput").ap()
    out = nc.dram_tensor("out", [BPC, N, QD], F32, kind="ExternalOutput").ap()

    with tile.TileContext(nc) as tc:
        with tc.tile_pool(name="wpool", bufs=1) as wpool, \
             tc.tile_pool(name="stage", bufs=1) as stage, \
             tc.tile_pool(name="xpool", bufs=2) as xpool, \
             tc.tile_pool(name="kvpool", bufs=2) as kvpool, \
             tc.tile_pool(name="bigpool", bufs=1) as bigpool, \
             tc.tile_pool(name="appool", bufs=2) as appool, \
             tc.tile_pool(name="opool", bufs=2) as opool, \
             tc.tile_pool(name="pp", bufs=2, space="PSUM") as pp:

            # ---------------- weight prep (bf16 casts) ----------------
            def load_cast(dram_ap, kt_count, mdim, name):
                st = stage.tile([128, kt_count, mdim], F32, tag="wstage",
                                name=f"st_{name}")
                nc.gpsimd.dma_start(
                    out=st[:],
                    in_=dram_ap.rearrange("(k p) m -> p k m", p=128))
                wbf = wpool.tile([128, kt_count, mdim], BF16, name=f"w_{name}")
                nc.vector.tensor_copy(wbf[:], st[:])
                return wbf

            wk = load_cast(Wk, 8, ID, "wk")
            wv = load_cast(Wv, 8, ID, "wv")
            wkip = load_cast(Wk_ip, 8, ID, "wkip")
            wvip = load_cast(Wv_ip, 8, ID, "wvip")
            wo = load_cast(Wo, 4, QD, "wo")

            # Wq [320, 512]: k-tiles 128/128/64
            st_wq = stage.tile([128, 3, ID], F32, tag="wstage")
            nc.gpsimd.dma_start(
                out=st_wq[:, 0:2, :],
                in_=Wq[0:256, :].rearrange("(k p) m -> p k m", p=128))
            nc.gpsimd.dma_start(out=st_wq[0:64, 2, :], in_=Wq[256:320, :])
            wq = wpool.tile([128, 3, ID], BF16)
            nc.vector.tensor_copy(wq[:, 0:2, :], st_wq[:, 0:2, :])
            nc.vector.tensor_copy(wq[0:64, 2, :], st_wq[0:64, 2, :])

            bo_f32 = wpool.tile([1, QD], F32)
            nc.scalar.dma_start(out=bo_f32[:], in_=bo[None, :])
            bo_bf = wpool.tile([1, QD], BF16)
            nc.vector.tensor_copy(bo_bf[:], bo_f32[:])
            ones1 = wpool.tile([1, 128], BF16)
            nc.gpsimd.memset(ones1[:], 1.0)

            ts_sb = wpool.tile([1, 1], F32)
            nc.scalar.dma_start(out=ts_sb[:], in_=tscale[:, None])
            is_sb = wpool.tile([1, 1], F32)
            nc.scalar.dma_start(out=is_sb[:], in_=iscale[:, None])
            ts_col = wpool.tile([128, 1], F32)
            nc.gpsimd.partition_broadcast(ts_col[:], ts_sb[:])
            is_col = wpool.tile([128, 1], F32)
            nc.gpsimd.partition_broadcast(is_col[:], is_sb[:])

            for b in range(BPC):
                # ---------------- context -> K^T, V ----------------
                ctx_f32 = kvpool.tile([128, CD], F32)
                nc.gpsimd.memset(ctx_f32[:], 0.0)
                nc.scalar.dma_start(out=ctx_f32[0:TXT, :], in_=ctx[b, 0:TXT, :])
                nc.scalar.dma_start(out=ctx_f32[IMG0:KSPAN, :],
                                    in_=ctx[b, TXT:93, :])
                ctx_bf = kvpool.tile([128, CD], BF16)
                nc.vector.tensor_copy(ctx_bf[:], ctx_f32[:])
                ctxt = kvpool.tile([128, 8, 128], BF16)
                nc.sync.dma_start(out=ctxt[:], in_=ctx_bf[:], transpose=True)

                # K^T  [512 (4 m-tiles), keys]
                psum_kt = pp.tile([128, 4, 128], F32, tag="proj", bufs=1)
                for m in range(4):
                    for k in range(8):
                        nc.tensor.matmul(
                            psum_kt[:, m, 0:TXT],
                            wk[:, k, 128 * m:128 * (m + 1)],
                            ctxt[:, k, 0:TXT],
                            start=(k == 0), stop=(k == 7))
                for m in range(4):
                    for k in range(8):
                        nc.tensor.matmul(
                            psum_kt[:, m, IMG0:KSPAN],
                            wkip[:, k, 128 * m:128 * (m + 1)],
                            ctxt[:, k, IMG0:KSPAN],
                            start=(k == 0), stop=(k == 7))
                kt = kvpool.tile([128, 4, 128], BF16)
                nc.gpsimd.memset(kt[:], 0.0)
                nc.vector.tensor_copy(kt[:, :, 0:TXT], psum_kt[:, :, 0:TXT])
                nc.vector.tensor_copy(kt[:, :, IMG0:KSPAN],
                                      psum_kt[:, :, IMG0:KSPAN])

                # V [keys, 512]
                psum_vt = pp.tile([128, ID], F32, tag="proj", bufs=1)
                for k in range(8):
                    nc.tensor.matmul(
                        psum_vt[0:TXT, :], ctxt[:, k, 0:TXT], wv[:, k, :],
                        start=(k == 0), stop=(k == 7))
                psum_vi = pp.tile([128, ID], F32, tag="proj", bufs=1)
                for k in range(8):
                    nc.tensor.matmul(
                        psum_vi[0:16, :], ctxt[:, k, IMG0:KSPAN], wvip[:, k, :],
                        start=(k == 0), stop=(k == 7))
                v_sb = kvpool.tile([128, ID], BF16)
                nc.gpsimd.memset(v_sb[:], 0.0)
                nc.vector.tensor_copy(v_sb[0:TXT, :], psum_vt[0:TXT, :])
                v_img = kvpool.tile([16, ID], BF16)
                nc.vector.tensor_copy(v_img[:], psum_vi[0:16, :])
                nc.scalar.dma_start(out=v_sb[IMG0:KSPAN, :], in_=v_img[:])

                # ---------------- x -> x^T (bf16), 4-chunk batches --------
                xt = bigpool.tile([128, NCH, 3, 128], BF16, tag="xtcomb",
                                  name="xt")
                for g in range(NG):
                    x4_f32 = xpool.tile([128, 4, QD], F32)
                    nc.scalar.dma_start(
                        out=x4_f32[:],
                        in_=x[b, 512 * g:512 * (g + 1), :]
                            .rearrange("(j p) d -> p j d", p=128))
                    x4_bf = xpool.tile([128, 4, 384], BF16)
                    for j in range(4):
                        nc.vector.tensor_copy(x4_bf[:, j, 0:QD], x4_f32[:, j, :])
                    nc.sync.dma_start(
                        out=xt[:, 4 * g:4 * (g + 1), :, :],
                        in_=x4_bf.rearrange("p j m -> p (j m)"),
                        transpose=True)

                # ---------------- Q^T = Wq.T @ x^T ----------------
                qt = bigpool.tile([128, 4, N], BF16, tag="qt")
                for m in range(4):
                    for n in range(8):
                        psum_q = pp.tile([128, 512], F32, tag="qproj", bufs=2)
                        for ki, kp in enumerate((128, 128, 64)):
                            nc.tensor.matmul(
                                psum_q[:],
                                wq[0:kp, ki, 128 * m:128 * (m + 1)],
                                xt[0:kp, 4 * n:4 * (n + 1), ki, :],
                                start=(ki == 0), stop=(ki == 2))
                        nc.scalar.activation(
                            qt[:, m, 512 * n:512 * (n + 1)], psum_q[:], AF.Copy)

                # ---------------- attention ----------------
                comb = bigpool.tile([128, 4, N], BF16, tag="xtcomb", name="comb")
                dsum = appool.tile([128, 8, 2, NCH], F32, tag="dsum", bufs=1)
                rsum = appool.tile([128, 8, 2, NCH], F32, tag="rsum", bufs=1)
                for g in range(NG):
                    probs = appool.tile([128, 8, 4, 128], BF16, tag="probs")
                    for h in range(H):
                        hp, hh = h // 2, h % 2
                        psum_s = pp.tile([128, 4, 128], F32, tag="sim", bufs=2)
                        for c4 in range(4):
                            tok = 128 * (4 * g + c4)
                            nc.tensor.matmul(
                                psum_s[:, c4, 0:KSPAN],
                                qt[64 * hh:64 * (hh + 1), hp, tok:tok + 128],
                                kt[64 * hh:64 * (hh + 1), hp, 0:KSPAN],
                                start=True, stop=True)
                        nc.scalar.activation(
                            probs[:, h, :, 0:KSPAN],
                            psum_s[:, :, 0:KSPAN], AF.Exp, scale=SCALE)
                        nc.vector.reduce_sum(
                            out=dsum[:, h, 0, 4 * g:4 * (g + 1)],
                            in_=probs[:, h, :, 0:TXT], axis=X_AX)
                        nc.vector.reduce_sum(
                            out=dsum[:, h, 1, 4 * g:4 * (g + 1)],
                            in_=probs[:, h, :, IMG0:KSPAN], axis=X_AX)
                        nc.vector.reciprocal(
                            rsum[:, h, :, 4 * g:4 * (g + 1)],
                            dsum[:, h, :, 4 * g:4 * (g + 1)])
                        nc.vector.scalar_tensor_tensor(
                            out=probs[:, h, :, 0:TXT],
                            in0=probs[:, h, :, 0:TXT],
                            scalar=ts_col[:, 0:1],
                            in1=rsum[:, h, 0, 4 * g:4 * (g + 1)][:, :, None]
                                .broadcast_to([128, 4, TXT]),
                            op0=ALU.mult, op1=ALU.mult)
                        nc.vector.scalar_tensor_tensor(
                            out=probs[:, h, :, IMG0:KSPAN],
                            in0=probs[:, h, :, IMG0:KSPAN],
                            scalar=is_col[:, 0:1],
                            in1=rsum[:, h, 1, 4 * g:4 * (g + 1)][:, :, None]
                                .broadcast_to([128, 4, IMG]),
                            op0=ALU.mult, op1=ALU.mult)
                    probsT = appool.tile([128, 32, 128], BF16, tag="probsT", bufs=1)
                    nc.sync.dma_start(
                        out=probsT[:],
                        in_=probs.rearrange("p h c k -> p (h c k)"),
                        transpose=True)
                    for hp in range(4):
                        psum_pv = pp.tile([128, 512], F32, tag="pv", bufs=2)
                        for hh in range(2):
                            h = 2 * hp + hh
                            nc.tensor.matmul(
                                psum_pv[64 * hh:64 * (hh + 1), :],
                                v_sb[0:KSPAN, 64 * h:64 * (h + 1)],
                                probsT[0:KSPAN, 4 * h:4 * (h + 1), :],
                                start=True, stop=True)
                        if hp % 2 == 0:
                            nc.vector.tensor_copy(
                                comb[:, hp, 512 * g:512 * (g + 1)], psum_pv[:])
                        else:
                            nc.scalar.activation(
                                comb[:, hp, 512 * g:512 * (g + 1)], psum_pv[:],
                                AF.Copy)

                # ---------------- out projection ----------------
                for g in range(NG):
                    out4 = opool.tile([128, 4, QD], F32)
                    for j in range(4):
                        c = 4 * g + j
                        psum_o_full = pp.tile([128, 512], F32, tag="qproj", bufs=2,
                                              name="psum_o")
                        psum_o = psum_o_full[:, 0:QD]
                        for k in range(4):
                            nc.tensor.matmul(
                                psum_o[:],
                                comb[:, k, 128 * c:128 * (c + 1)],
                                wo[:, k, :],
                                start=(k == 0), stop=False)
                        nc.tensor.matmul(
                            psum_o[:], ones1[:, :], bo_bf[:, :],
                            start=False, stop=True)
                        nc.scalar.activation(out4[:, j, :], psum_o[:], AF.Copy)
                    nc.scalar.dma_start(
                        out=out[b, 512 * g:512 * (g + 1), :]
                            .rearrange("(j p) d -> p j d", p=128),
                        in_=out4[:])

    nc.compile()
    return nc


def _get_nc():
    global _NC_CACHE
    if _NC_CACHE is None:
        _NC_CACHE = _build_nc()
    return _NC_CACHE


def kernel(x, context, Wq, Wk, Wv, Wk_ip, Wv_ip, Wo, bo, text_scale, img_scale):
    x = np.ascontiguousarray(np.asarray(x, dtype=np.float32))
    context = np.ascontiguousarray(np.asarray(context, dtype=np.float32))
    shared = {
        "Wq": np.asarray(Wq, np.float32), "Wk": np.asarray(Wk, np.float32),
        "Wv": np.asarray(Wv, np.float32), "Wk_ip": np.asarray(Wk_ip, np.float32),
        "Wv_ip": np.asarray(Wv_ip, np.float32), "Wo": np.asarray(Wo, np.float32),
        "bo": np.asarray(bo, np.float32),
        "text_scale": np.asarray(text_scale, np.float32),
        "img_scale": np.asarray(img_scale, np.float32),
    }
    nc = _get_nc()
    in_maps = []
    for c in range(N_CORES):
        m = dict(shared)
        m["x"] = x[BPC * c:BPC * (c + 1)]
        m["context"] = context[BPC * c:BPC * (c + 1)]
        in_maps.append(m)
    res = run_bass_kernel_spmd(nc, in_maps, core_ids=list(range(N_CORES)))
    return np.concatenate([res.results[c]["out"] for c in range(N_CORES)], axis=0)
